# revision 11
# baseline (speedup 1.0000x reference)
"""Fused BN(inference)+ReLU -> 1x1 conv (512->256) -> 2x2 avgpool on 8 TRN2 cores.

Full inputs in, full output out. Data-parallel over batch (16 -> 2 per core),
BN params + conv weights replicated.

Math folding (host side, tiny):
  s = bn_weight / sqrt(bn_var + eps)            [512]
  t = bn_bias - bn_mean * s                     [512]
  y = relu(s * x + t)                           (one ACT op per channel tile)
  avgpool2x2(W @ y) == (0.25 * W) @ sumpool2x2(y)   (pool before matmul: 4x
                                                     fewer matmul FLOPs)
  wt = 0.25 * W.T                               [512, 256] (lhsT layout)

v2 structure (from trace analysis of v1 at ~57us):
  - The x stream runs at ~380 GB/s on the sync HWDGE queue and is the hard
    floor (12.8 MB/core fp32 in).  Everything else must hide under it.
  - v1 lost ~7us after the last x byte: 56-row chunk granularity meant the
    last chunk's RELU(3us) -> adds -> matmuls -> PSUM copy -> out DMA chain
    drained serially.  v2 shrinks the tail chunks (28/14 rows) so the
    post-stream drain is ~3us.
  - Post-ReLU pipeline is bf16: halves DVE add cost, 1 cyc/row matmuls
    (vs 4 for fp32), halves wt HBM bytes.  PSUM accumulates fp32; the
    fp32 output path is unchanged.  Measured rel err stays ~1e-3 vs the
    2e-2 gate.
  - s,t ship as one [128, 8] tensor (one descriptor set instead of two
    128x16B sprays); wt ships bf16.
  - Output DMAs stay on the sync HWDGE queue (SWDGE/gpsimd DMAs poison
    SDMA engines 7/15 via descriptor-ring port contention, which delays
    every x-chunk completion semaphore).  To avoid head-of-line blocking
    of the x stream, all out DMAs are EMITTED after the last x-chunk DMA,
    so the SP FIFO is [all x] then [outs]; each out waits only on its
    staging copy.  Staging copies are split ACT/DVE so the two pieces of
    a close event stage in parallel.
"""

import copy as _copy

import numpy as np

import bass_rust
import concourse.bass as bass
import concourse.mybir as mybir
import concourse.tile as tile_mod
from concourse.bass_utils import run_bass_kernel_spmd

EPS = 1e-5

B, C_IN, C_OUT, H, W = 16, 512, 256, 56, 56
N_CORES = 8
B_PC = B // N_CORES          # batches per core
HW = H * W                   # 3136
HWP = (H // 2) * (W // 2)    # 784 pooled spatial
K_TILES = C_IN // 128        # 4
M_TILES = C_OUT // 128       # 2
N_CHUNK = HWP // 2           # 392 (fits one PSUM bank)

_DT = mybir.dt.float32
_BF = mybir.dt.bfloat16


# This walrus build enforces per-instruction sync-wait caps that Tile's
# add_semaphores pass does not respect: CTRL-type instructions (Drain, NoOp)
# take no sem-ge waits at all, EventSemaphore takes at most 2, and every
# other instruction takes at most 1. Post-pass: hoist excess waits onto
# EventSemaphore carrier instructions inserted just before the owning
# instruction on the same engine (same blocking semantics - the carrier
# blocks the engine's sequencer until its waits pass).
_CTRL_OPS = ("InstDrain", "InstNoOp")


def _hoist_excess_waits(nc):
    ev_counter = [0]

    def make_carrier(engine, waits):
        ev_counter[0] += 1
        return mybir.InstEventSemaphore(
            name=f"EVHOIST-{ev_counter[0]}",
            engine=engine,
            ins=[],
            outs=[],
            sync_info=bass_rust.SyncInfo(on_wait=waits, on_update=[]),
        )

    new_module = _copy.replace(nc.m, functions=[])
    for function in nc.m.functions:
        new_function = _copy.replace(function, blocks=[])
        new_function.set_allocations_from_list(function.allocations)
        for block in function.blocks:
            new_insts = []
            for ins in block.instructions:
                si = ins.sync_info
                waits = list(si.on_wait) if si is not None else []
                opname = type(ins).__name__
                if opname in _CTRL_OPS:
                    keep = [w for w in waits if w.wait_mode != "sem-ge-imm"]
                    excess = [w for w in waits if w.wait_mode == "sem-ge-imm"]
                else:
                    limit = 2 if opname == "InstEventSemaphore" else 1
                    keep, excess = waits[:limit], waits[limit:]
                if excess:
                    for i in range(0, len(excess), 2):
                        new_insts.append(make_carrier(ins.engine, excess[i : i + 2]))
                    si.on_wait = keep
                new_insts.append(ins)
            new_function.blocks.append(_copy.replace(block, instructions=new_insts))
        new_module.functions.append(new_function)
    nc.m = new_module


# Per-(b, k) row split.  nrows multiples of 14 (14 rows -> 196 pooled cols).
# Big chunks mid-stream for DMA efficiency; small chunks at the global tail
# so the post-stream RELU->add->matmul->copy->DMA drain is short.
def _chunk_plan():
    plan = {}
    for b in range(B_PC):
        for k in range(K_TILES):
            if b == 0 and k == 0:
                plan[(b, k)] = [(0, 28), (28, 28)]       # fast pipeline fill
            elif b == B_PC - 1 and k == K_TILES - 2:
                plan[(b, k)] = [(0, 28), (28, 28)]       # overlap tail compute
            elif b == B_PC - 1 and k == K_TILES - 1:
                plan[(b, k)] = [(0, 28), (28, 14), (42, 14)]  # short drain
            else:
                plan[(b, k)] = [(0, 56)]
    return plan


def build_bass():
    nc = bass.Bass()

    # Params come pre-transposed from the host into partition-major layouts so
    # their DMAs are fully contiguous.  s and t ride in one [128, 8] tensor.
    x_d = nc.dram_tensor("x", [B_PC, C_IN, H, W], _DT, kind="ExternalInput")
    st_d = nc.dram_tensor("st", [128, 2 * K_TILES], _DT, kind="ExternalInput")
    wt_d = nc.dram_tensor(
        "wt", [128, K_TILES, C_OUT], _BF, kind="ExternalInput"
    )
    out_d = nc.dram_tensor(
        "out", [B_PC, C_OUT, H // 2, W // 2], _DT, kind="ExternalOutput"
    )

    plan = _chunk_plan()

    with tile_mod.TileContext(nc) as tc:
        with (
            tc.tile_pool(name="const", bufs=1) as cpool,
            tc.tile_pool(name="xs", bufs=8) as xpool,
            tc.tile_pool(name="ys", bufs=4) as ypool,
            tc.tile_pool(name="us", bufs=4) as upool,
            tc.tile_pool(name="ps", bufs=4) as ppool,
            tc.tile_pool(name="os", bufs=8) as opool,
            tc.tile_pool(name="psum", bufs=8, space="PSUM") as pspool,
        ):
            # First x half-chunk goes FIRST on the sync FIFO (longest consumer
            # chain), then the tiny st, then the second half, then wt (only
            # needed at the first matmul, ~4us later).
            x00 = xpool.tile([128, 28 * W], _DT, tag="x", name="x_0_0_0")
            nc.sync.dma_start(
                out=x00[:],
                in_=x_d[0, 0:128, 0:28].rearrange("ch h w -> ch (h w)"),
            )
            st_sb = cpool.tile([128, 2 * K_TILES], _DT)
            nc.sync.dma_start(out=st_sb[:], in_=st_d[:])
            s_sb = st_sb[:, 0:K_TILES]
            t_sb = st_sb[:, K_TILES : 2 * K_TILES]
            x01 = xpool.tile([128, 28 * W], _DT, tag="x", name="x_0_0_1")
            nc.sync.dma_start(
                out=x01[:],
                in_=x_d[0, 0:128, 28:56].rearrange("ch h w -> ch (h w)"),
            )
            wt_sb = cpool.tile([128, K_TILES, C_OUT], _BF)
            nc.sync.dma_start(out=wt_sb[:], in_=wt_d[:])
            # Trigger the lazy ACT Relu table load now, off the critical path
            warm = cpool.tile([1, 1], _DT)
            nc.scalar.activation(
                warm[:], st_sb[0:1, 0:1], mybir.ActivationFunctionType.Relu
            )

            pre = {(0, 0, 0): x00, (0, 0, 28): x01}

            def emit_chunk(b, k, row0, nrows, psums, first_k, last_k):
                """Rows [row0, row0+nrows) of k-slice k: DMA -> BN+ReLU(bf16)
                -> 2x2 sum-pool -> bf16 matmul into psum pieces."""
                c = row0 // 14
                hc = nrows * W
                x_t = pre.get((b, k, row0))
                if x_t is None:
                    x_t = xpool.tile(
                        [128, hc], _DT, tag="x", name=f"x_{b}_{k}_{c}"
                    )
                    nc.sync.dma_start(
                        out=x_t[:],
                        in_=x_d[
                            b,
                            k * 128 : (k + 1) * 128,
                            row0 : row0 + nrows,
                        ].rearrange("ch h w -> ch (h w)"),
                    )
                y_t = ypool.tile([128, hc], _BF, tag="y", name=f"y_{b}_{k}_{c}")
                nc.scalar.activation(
                    y_t[:],
                    x_t[:],
                    mybir.ActivationFunctionType.Relu,
                    bias=t_sb[:, k : k + 1],
                    scale=s_sb[:, k : k + 1],
                )
                # H-pairs first: operands are contiguous 56-elem runs
                u_t = upool.tile(
                    [128, hc // 2], _BF, tag="u", name=f"u_{b}_{k}_{c}"
                )
                yv = y_t[:].rearrange("p (h two w) -> p h two w", two=2, w=W)
                nc.vector.tensor_add(u_t[:], yv[:, :, 0, :], yv[:, :, 1, :])
                # then W-pairs
                p_t = ppool.tile(
                    [128, hc // 4], _BF, tag="p", name=f"p_{b}_{k}_{c}"
                )
                uv = u_t[:].rearrange("p (a two) -> p a two", two=2)
                nc.vector.tensor_add(p_t[:], uv[:, :, 0], uv[:, :, 1])
                # map this chunk's pooled columns onto psum n-chunk pieces
                pooled0 = (row0 // 2) * (W // 2)  # global pooled col offset
                pooled_w = (nrows // 2) * (W // 2)
                for m in range(M_TILES):
                    off = 0
                    while off < pooled_w:
                        g = pooled0 + off  # global pooled col
                        n = g // N_CHUNK
                        col = g % N_CHUNK
                        width = min(N_CHUNK - col, pooled_w - off)
                        if first_k and (m, n) not in psums:
                            psums[(m, n)] = pspool.tile(
                                [128, N_CHUNK],
                                _DT,
                                tag="psum",
                                name=f"psum_{b}_{m}_{n}",
                            )
                        nc.tensor.matmul(
                            psums[(m, n)][:, col : col + width],
                            wt_sb[:, k, m * 128 : (m + 1) * 128],
                            p_t[:, off : off + width],
                            start=(first_k and col == 0),
                            stop=(last_k and col + width == N_CHUNK),
                            skip_group_check=True,
                        )
                        off += width

            out_v = out_d[:].rearrange("bb o h w -> bb o (h w)")
            deferred_outs = []

            def out_slice(b, m, n):
                return out_v[
                    b,
                    m * 128 : (m + 1) * 128,
                    n * N_CHUNK : (n + 1) * N_CHUNK,
                ]

            def stage(b, m, n, psums, copy_eng, dma_eng):
                # PSUM -> SBUF (DMA can't read PSUM).  dma_eng "sync" outs
                # are deferred past the last x-chunk DMA so the SP FIFO
                # stays [all x][all outs]; "act" outs issue inline on the
                # scalar queue right after their same-engine copy.
                o_t = opool.tile(
                    [128, N_CHUNK], _DT, tag="o", name=f"o_{b}_{m}_{n}"
                )
                if copy_eng == "act":
                    nc.scalar.copy(o_t[:], psums[(m, n)][:])
                else:
                    nc.vector.tensor_copy(o_t[:], psums[(m, n)][:])
                if dma_eng == "act":
                    nc.scalar.dma_start(out=out_slice(b, m, n), in_=o_t[:])
                else:
                    deferred_outs.append((b, m, n, o_t))

            for b in range(B_PC):
                last_b = b == B_PC - 1
                psums = {}
                for k in range(K_TILES):
                    first_k = k == 0
                    last_k = k == K_TILES - 1
                    for row0, nrows in plan[(b, k)]:
                        emit_chunk(b, k, row0, nrows, psums, first_k, last_k)
                if not last_b:
                    # Mid-stream batch: stage on DVE (the ACT RELU chain is
                    # the tail critical path - keep it clean), ship on sync.
                    for m in range(M_TILES):
                        for n in range(2):
                            stage(b, m, n, psums, "dve", "sync")
                else:
                    # Final batch: pair ACT+DVE copies and scalar+sync
                    # queues so the four pieces drain in parallel.  ACT is
                    # idle once its last RELU retires, so the m==0 pieces
                    # ride the scalar engine end to end.
                    for n in range(2):
                        stage(b, 0, n, psums, "act", "act")
                        stage(b, 1, n, psums, "dve", "sync")

            for b, m, n, o_t in deferred_outs:
                nc.sync.dma_start(out=out_slice(b, m, n), in_=o_t[:])
    _hoist_excess_waits(nc)
    return nc


_NC_CACHE = None


def _get_nc():
    global _NC_CACHE
    if _NC_CACHE is None:
        _NC_CACHE = build_bass()
    return _NC_CACHE


def _to_bf16(a):
    """Round-to-nearest-even fp32 -> bf16, returned as ml_dtypes.bfloat16."""
    import ml_dtypes

    return a.astype(ml_dtypes.bfloat16)


def _prep_host(bn_weight, bn_bias, bn_mean, bn_var, conv_weight):
    s = (bn_weight / np.sqrt(bn_var + EPS)).astype(np.float32)
    t = (bn_bias - bn_mean * s).astype(np.float32)
    wt = (0.25 * conv_weight.T).astype(np.float32)  # [C_IN, C_OUT]
    # partition-major layouts: [128, 2K] for s|t, [128, K, C_OUT] for wt
    s2 = s.reshape(K_TILES, 128).T
    t2 = t.reshape(K_TILES, 128).T
    st = np.ascontiguousarray(np.concatenate([s2, t2], axis=1))
    wt2 = np.ascontiguousarray(
        wt.reshape(K_TILES, 128, C_OUT).transpose(1, 0, 2)
    )
    return st, _to_bf16(wt2)


def _install_ntff_hook():
    # The agent image's antenv lacks axon_hooks; synthesize it from the boot
    # shim's ctypes factory so trace=True captures NTFF profiles.
    import sys
    import types

    try:
        import antenv.axon_hooks  # noqa: F401

        return
    except ImportError:
        pass
    from trn_agent_boot.trn_boot import _ntff_profile_via_ctypes

    hook = _ntff_profile_via_ctypes("/opt/axon/libaxon_pjrt.so")
    mod = types.ModuleType("antenv.axon_hooks")
    store = {"h": hook}
    mod.get_axon_ntff_profile_hook = lambda: store["h"]
    mod.set_axon_ntff_profile_hook = lambda h: store.__setitem__("h", h)
    import antenv

    antenv.axon_hooks = mod
    sys.modules["antenv.axon_hooks"] = mod


def kernel(x, bn_weight, bn_bias, bn_mean, bn_var, conv_weight, _trace=False):
    if _trace:
        _install_ntff_hook()
    x = np.asarray(x, dtype=np.float32)
    st, wt = _prep_host(
        np.asarray(bn_weight, dtype=np.float32),
        np.asarray(bn_bias, dtype=np.float32),
        np.asarray(bn_mean, dtype=np.float32),
        np.asarray(bn_var, dtype=np.float32),
        np.asarray(conv_weight, dtype=np.float32),
    )
    in_maps = [
        {"x": np.ascontiguousarray(x[c * B_PC : (c + 1) * B_PC]), "st": st, "wt": wt}
        for c in range(N_CORES)
    ]
    nc = _get_nc()
    res = run_bass_kernel_spmd(
        nc, in_maps, core_ids=list(range(N_CORES)), trace=_trace
    )
    out = np.concatenate([res.results[c]["out"] for c in range(N_CORES)], axis=0)
    if _trace:
        return out, res
    return out


# revision 12
# speedup vs baseline: 1.0571x; 1.0571x over previous
"""Fused BN(inference)+ReLU -> 1x1 conv (512->256) -> 2x2 avgpool on 8 TRN2 cores.

Full inputs in, full output out. Data-parallel over batch (16 -> 2 per core),
BN params + conv weights replicated.

Math folding (host side, tiny):
  s = bn_weight / sqrt(bn_var + eps)            [512]
  t = bn_bias - bn_mean * s                     [512]
  y = relu(s * x + t)                           (one ACT op per channel tile)
  avgpool2x2(W @ y) == (0.25 * W) @ sumpool2x2(y)   (pool before matmul: 4x
                                                     fewer matmul FLOPs)
  wt = 0.25 * W.T                               [512, 256] (lhsT layout)

v2 structure (from trace analysis of v1 at ~57us):
  - The x stream runs at ~380 GB/s on the sync HWDGE queue and is the hard
    floor (12.8 MB/core fp32 in).  Everything else must hide under it.
  - v1 lost ~7us after the last x byte: 56-row chunk granularity meant the
    last chunk's RELU(3us) -> adds -> matmuls -> PSUM copy -> out DMA chain
    drained serially.  v2 shrinks the tail chunks (28/14 rows) so the
    post-stream drain is ~3us.
  - Post-ReLU pipeline is bf16: halves DVE add cost, 1 cyc/row matmuls
    (vs 4 for fp32), halves wt HBM bytes.  PSUM accumulates fp32; the
    fp32 output path is unchanged.  Measured rel err stays ~1e-3 vs the
    2e-2 gate.
  - s,t ship as one [128, 8] tensor (one descriptor set instead of two
    128x16B sprays); wt ships bf16.
  - Output DMAs stay on the sync HWDGE queue (SWDGE/gpsimd DMAs poison
    SDMA engines 7/15 via descriptor-ring port contention, which delays
    every x-chunk completion semaphore).  To avoid head-of-line blocking
    of the x stream, all out DMAs are EMITTED after the last x-chunk DMA,
    so the SP FIFO is [all x] then [outs]; each out waits only on its
    staging copy.  Staging copies are split ACT/DVE so the two pieces of
    a close event stage in parallel.
"""

import copy as _copy

import numpy as np

import bass_rust
import concourse.bass as bass
import concourse.mybir as mybir
import concourse.tile as tile_mod
from concourse.bass_utils import run_bass_kernel_spmd

EPS = 1e-5

B, C_IN, C_OUT, H, W = 16, 512, 256, 56, 56
N_CORES = 8
B_PC = B // N_CORES          # batches per core
HW = H * W                   # 3136
HWP = (H // 2) * (W // 2)    # 784 pooled spatial
K_TILES = C_IN // 128        # 4
M_TILES = C_OUT // 128       # 2
N_CHUNK = HWP // 2           # 392 (fits one PSUM bank)

_DT = mybir.dt.float32
_BF = mybir.dt.bfloat16


# This walrus build enforces per-instruction sync-wait caps that Tile's
# add_semaphores pass does not respect: CTRL-type instructions (Drain, NoOp)
# take no sem-ge waits at all, EventSemaphore takes at most 2, and every
# other instruction takes at most 1. Post-pass: hoist excess waits onto
# EventSemaphore carrier instructions inserted just before the owning
# instruction on the same engine (same blocking semantics - the carrier
# blocks the engine's sequencer until its waits pass).
_CTRL_OPS = ("InstDrain", "InstNoOp")


def _hoist_excess_waits(nc):
    ev_counter = [0]

    def make_carrier(engine, waits):
        ev_counter[0] += 1
        return mybir.InstEventSemaphore(
            name=f"EVHOIST-{ev_counter[0]}",
            engine=engine,
            ins=[],
            outs=[],
            sync_info=bass_rust.SyncInfo(on_wait=waits, on_update=[]),
        )

    new_module = _copy.replace(nc.m, functions=[])
    for function in nc.m.functions:
        new_function = _copy.replace(function, blocks=[])
        new_function.set_allocations_from_list(function.allocations)
        for block in function.blocks:
            new_insts = []
            for ins in block.instructions:
                si = ins.sync_info
                waits = list(si.on_wait) if si is not None else []
                opname = type(ins).__name__
                if opname in _CTRL_OPS:
                    keep = [w for w in waits if w.wait_mode != "sem-ge-imm"]
                    excess = [w for w in waits if w.wait_mode == "sem-ge-imm"]
                else:
                    limit = 2 if opname == "InstEventSemaphore" else 1
                    keep, excess = waits[:limit], waits[limit:]
                if excess:
                    for i in range(0, len(excess), 2):
                        new_insts.append(make_carrier(ins.engine, excess[i : i + 2]))
                    si.on_wait = keep
                new_insts.append(ins)
            new_function.blocks.append(_copy.replace(block, instructions=new_insts))
        new_module.functions.append(new_function)
    nc.m = new_module


# Per-(b, k) row split.  nrows multiples of 14 (14 rows -> 196 pooled cols).
# Big chunks mid-stream for DMA efficiency; small chunks at the global tail
# so the post-stream RELU->add->matmul->copy->DMA drain is short.
def _chunk_plan():
    plan = {}
    for b in range(B_PC):
        for k in range(K_TILES):
            if b == 0 and k == 0:
                plan[(b, k)] = [(0, 28), (28, 28)]       # fast pipeline fill
            elif b == B_PC - 1 and k == K_TILES - 1:
                plan[(b, k)] = [(0, 28), (28, 14), (42, 14)]  # short drain
            else:
                plan[(b, k)] = [(0, 56)]
    return plan


def build_bass():
    nc = bass.Bass()

    # Params come pre-transposed from the host into partition-major layouts so
    # their DMAs are fully contiguous.  s and t ride in one [128, 8] tensor.
    x_d = nc.dram_tensor("x", [B_PC, C_IN, H, W], _DT, kind="ExternalInput")
    st_d = nc.dram_tensor("st", [128, 2 * K_TILES], _DT, kind="ExternalInput")
    wt_d = nc.dram_tensor(
        "wt", [128, K_TILES, C_OUT], _BF, kind="ExternalInput"
    )
    out_d = nc.dram_tensor(
        "out", [B_PC, C_OUT, H // 2, W // 2], _DT, kind="ExternalOutput"
    )

    plan = _chunk_plan()

    with tile_mod.TileContext(nc) as tc:
        with (
            tc.tile_pool(name="const", bufs=1) as cpool,
            tc.tile_pool(name="xs", bufs=8) as xpool,
            tc.tile_pool(name="ys", bufs=4) as ypool,
            tc.tile_pool(name="us", bufs=4) as upool,
            tc.tile_pool(name="ps", bufs=4) as ppool,
            tc.tile_pool(name="os", bufs=8) as opool,
            tc.tile_pool(name="psum", bufs=8, space="PSUM") as pspool,
        ):
            # First x half-chunk goes FIRST on the sync FIFO (longest consumer
            # chain), then the tiny st, then the second half, then wt (only
            # needed at the first matmul, ~4us later).
            x00 = xpool.tile([128, 28 * W], _DT, tag="x", name="x_0_0_0")
            nc.sync.dma_start(
                out=x00[:],
                in_=x_d[0, 0:128, 0:28].rearrange("ch h w -> ch (h w)"),
            )
            st_sb = cpool.tile([128, 2 * K_TILES], _DT)
            nc.sync.dma_start(out=st_sb[:], in_=st_d[:])
            s_sb = st_sb[:, 0:K_TILES]
            t_sb = st_sb[:, K_TILES : 2 * K_TILES]
            x01 = xpool.tile([128, 28 * W], _DT, tag="x", name="x_0_0_1")
            nc.sync.dma_start(
                out=x01[:],
                in_=x_d[0, 0:128, 28:56].rearrange("ch h w -> ch (h w)"),
            )
            wt_sb = cpool.tile([128, K_TILES, C_OUT], _BF)
            nc.sync.dma_start(out=wt_sb[:], in_=wt_d[:])
            # Trigger the lazy ACT Relu table load now, off the critical path
            warm = cpool.tile([1, 1], _DT)
            nc.scalar.activation(
                warm[:], st_sb[0:1, 0:1], mybir.ActivationFunctionType.Relu
            )

            pre = {(0, 0, 0): x00, (0, 0, 28): x01}

            def emit_chunk(b, k, row0, nrows, psums, first_k, last_k):
                """Rows [row0, row0+nrows) of k-slice k: DMA -> BN+ReLU(bf16)
                -> 2x2 sum-pool -> bf16 matmul into psum pieces."""
                c = row0 // 14
                hc = nrows * W
                x_t = pre.get((b, k, row0))
                if x_t is None:
                    x_t = xpool.tile(
                        [128, hc], _DT, tag="x", name=f"x_{b}_{k}_{c}"
                    )
                    nc.sync.dma_start(
                        out=x_t[:],
                        in_=x_d[
                            b,
                            k * 128 : (k + 1) * 128,
                            row0 : row0 + nrows,
                        ].rearrange("ch h w -> ch (h w)"),
                    )
                y_t = ypool.tile([128, hc], _BF, tag="y", name=f"y_{b}_{k}_{c}")
                nc.scalar.activation(
                    y_t[:],
                    x_t[:],
                    mybir.ActivationFunctionType.Relu,
                    bias=t_sb[:, k : k + 1],
                    scale=s_sb[:, k : k + 1],
                )
                # H-pairs first: operands are contiguous 56-elem runs
                u_t = upool.tile(
                    [128, hc // 2], _BF, tag="u", name=f"u_{b}_{k}_{c}"
                )
                yv = y_t[:].rearrange("p (h two w) -> p h two w", two=2, w=W)
                nc.vector.tensor_add(u_t[:], yv[:, :, 0, :], yv[:, :, 1, :])
                # then W-pairs
                p_t = ppool.tile(
                    [128, hc // 4], _BF, tag="p", name=f"p_{b}_{k}_{c}"
                )
                uv = u_t[:].rearrange("p (a two) -> p a two", two=2)
                nc.vector.tensor_add(p_t[:], uv[:, :, 0], uv[:, :, 1])
                # map this chunk's pooled columns onto psum n-chunk pieces
                pooled0 = (row0 // 2) * (W // 2)  # global pooled col offset
                pooled_w = (nrows // 2) * (W // 2)
                for m in range(M_TILES):
                    off = 0
                    while off < pooled_w:
                        g = pooled0 + off  # global pooled col
                        n = g // N_CHUNK
                        col = g % N_CHUNK
                        width = min(N_CHUNK - col, pooled_w - off)
                        if first_k and (m, n) not in psums:
                            psums[(m, n)] = pspool.tile(
                                [128, N_CHUNK],
                                _DT,
                                tag="psum",
                                name=f"psum_{b}_{m}_{n}",
                            )
                        nc.tensor.matmul(
                            psums[(m, n)][:, col : col + width],
                            wt_sb[:, k, m * 128 : (m + 1) * 128],
                            p_t[:, off : off + width],
                            start=(first_k and col == 0),
                            stop=(last_k and col + width == N_CHUNK),
                            skip_group_check=True,
                        )
                        off += width

            out_v = out_d[:].rearrange("bb o h w -> bb o (h w)")
            deferred_outs = []

            def out_slice(b, m, n):
                return out_v[
                    b,
                    m * 128 : (m + 1) * 128,
                    n * N_CHUNK : (n + 1) * N_CHUNK,
                ]

            def stage(b, m, n, psums, copy_eng, dma_eng):
                # PSUM -> SBUF (DMA can't read PSUM).  dma_eng "sync" outs
                # are deferred past the last x-chunk DMA so the SP FIFO
                # stays [all x][all outs]; "act" outs issue inline on the
                # scalar queue right after their same-engine copy.
                o_t = opool.tile(
                    [128, N_CHUNK], _DT, tag="o", name=f"o_{b}_{m}_{n}"
                )
                if copy_eng == "act":
                    nc.scalar.copy(o_t[:], psums[(m, n)][:])
                else:
                    nc.vector.tensor_copy(o_t[:], psums[(m, n)][:])
                if dma_eng == "act":
                    nc.scalar.dma_start(out=out_slice(b, m, n), in_=o_t[:])
                else:
                    deferred_outs.append((b, m, n, o_t))

            for b in range(B_PC):
                last_b = b == B_PC - 1
                psums = {}
                for k in range(K_TILES):
                    first_k = k == 0
                    last_k = k == K_TILES - 1
                    for row0, nrows in plan[(b, k)]:
                        emit_chunk(b, k, row0, nrows, psums, first_k, last_k)
                if not last_b:
                    # Mid-stream batch: stage on DVE (the ACT RELU chain is
                    # the tail critical path - keep it clean), ship on sync.
                    for m in range(M_TILES):
                        for n in range(2):
                            stage(b, m, n, psums, "dve", "sync")
                else:
                    # Final batch: pair ACT+DVE copies and scalar+sync
                    # queues so the four pieces drain in parallel.  ACT is
                    # idle once its last RELU retires, so the m==0 pieces
                    # ride the scalar engine end to end.
                    for n in range(2):
                        stage(b, 0, n, psums, "act", "act")
                        stage(b, 1, n, psums, "dve", "sync")

            for b, m, n, o_t in deferred_outs:
                nc.sync.dma_start(out=out_slice(b, m, n), in_=o_t[:])
    _hoist_excess_waits(nc)
    return nc


_NC_CACHE = None


def _get_nc():
    global _NC_CACHE
    if _NC_CACHE is None:
        _NC_CACHE = build_bass()
    return _NC_CACHE


def _to_bf16(a):
    """Round-to-nearest-even fp32 -> bf16, returned as ml_dtypes.bfloat16."""
    import ml_dtypes

    return a.astype(ml_dtypes.bfloat16)


def _prep_host(bn_weight, bn_bias, bn_mean, bn_var, conv_weight):
    s = (bn_weight / np.sqrt(bn_var + EPS)).astype(np.float32)
    t = (bn_bias - bn_mean * s).astype(np.float32)
    wt = (0.25 * conv_weight.T).astype(np.float32)  # [C_IN, C_OUT]
    # partition-major layouts: [128, 2K] for s|t, [128, K, C_OUT] for wt
    s2 = s.reshape(K_TILES, 128).T
    t2 = t.reshape(K_TILES, 128).T
    st = np.ascontiguousarray(np.concatenate([s2, t2], axis=1))
    wt2 = np.ascontiguousarray(
        wt.reshape(K_TILES, 128, C_OUT).transpose(1, 0, 2)
    )
    return st, _to_bf16(wt2)


def _install_ntff_hook():
    # The agent image's antenv lacks axon_hooks; synthesize it from the boot
    # shim's ctypes factory so trace=True captures NTFF profiles.
    import sys
    import types

    try:
        import antenv.axon_hooks  # noqa: F401

        return
    except ImportError:
        pass
    from trn_agent_boot.trn_boot import _ntff_profile_via_ctypes

    hook = _ntff_profile_via_ctypes("/opt/axon/libaxon_pjrt.so")
    mod = types.ModuleType("antenv.axon_hooks")
    store = {"h": hook}
    mod.get_axon_ntff_profile_hook = lambda: store["h"]
    mod.set_axon_ntff_profile_hook = lambda h: store.__setitem__("h", h)
    import antenv

    antenv.axon_hooks = mod
    sys.modules["antenv.axon_hooks"] = mod


def kernel(x, bn_weight, bn_bias, bn_mean, bn_var, conv_weight, _trace=False):
    if _trace:
        _install_ntff_hook()
    x = np.asarray(x, dtype=np.float32)
    st, wt = _prep_host(
        np.asarray(bn_weight, dtype=np.float32),
        np.asarray(bn_bias, dtype=np.float32),
        np.asarray(bn_mean, dtype=np.float32),
        np.asarray(bn_var, dtype=np.float32),
        np.asarray(conv_weight, dtype=np.float32),
    )
    in_maps = [
        {"x": np.ascontiguousarray(x[c * B_PC : (c + 1) * B_PC]), "st": st, "wt": wt}
        for c in range(N_CORES)
    ]
    nc = _get_nc()
    res = run_bass_kernel_spmd(
        nc, in_maps, core_ids=list(range(N_CORES)), trace=_trace
    )
    out = np.concatenate([res.results[c]["out"] for c in range(N_CORES)], axis=0)
    if _trace:
        return out, res
    return out
